# revision 19
# baseline (speedup 1.0000x reference)
"""Trainium2 Bass kernel for the branched (command-routed) control MLP.

Model: sfeat = tanh-MLP(spd) [B,128]; x = [rgb_feat | sfeat] [B,640];
per-row branch b = cmd[r] selects one of 7 MLPs 640->256->256->256 (sigmoid)
and 3 scalar heads (sigmoid/tanh/sigmoid).

Strategy (MoE routing): rows are sorted by cmd on the host and packed into
single-branch tiles (512 rows, plus one 256-row tile per core for branch
remainders); tiles are distributed over 8 NeuronCores.  Each tile's branch
weights are shipped per-tile (bf16), so the device program is identical on
every core (SPMD) while computing only the selected branch per row: 7x less
matmul work than the all-branch reference.

Device: software-pipelined across tiles (8 stages: DMA, speed-MLP x3,
branch layers x3, heads), so the in-order PE/ScalarE streams never stall on
the intra-tile matmul->sigmoid chain.  Branch layers run as K-accumulated
bf16 matmuls into fp32 PSUM with one [128,1024] sigmoid per layer per tile
(both 128-row H chunks in one ScalarE op); the 3 heads are one [K=256,M=3]
matmul whose fp32 pre-activations go back via an idle-VectorE copy + DMA.
The host adds head biases, applies the final sigmoid/tanh exactly, and
scatters tile outputs to original row order; padded rows are duplicates of
same-branch rows so duplicate scatters write identical values.

Two compiled variants: a fast bias-free path (all biases in this model's
init are zero) and a general path applying per-partition biases on ScalarE.
Measured on 8 axon trn2 cores: ~77 us HW exec (vs ~104 us for the same
mapping without software pipelining).
"""

import sys

if "/opt/trn_rl_repo" not in sys.path:
    sys.path.insert(0, "/opt/trn_rl_repo")

import math

import ml_dtypes
import numpy as np

B = 32768
D_RGB = 512
D_SPD = 128
D_IN = 640  # 5 * 128
H = 256
C = 7
N_CORES = 8
RT = 512  # rows per tile
KX = 4  # rgb K-chunks of 128

# packed weight column offsets (bf16 [128, NW] per tile)
OFF_W1 = 0  # 10 blocks of 128: (m*5+k)
OFF_W2 = 1280  # 4 blocks of 128: (m*2+k)
OFF_W3 = 1792  # 4 blocks of 128
OFF_H = 2304  # 2 blocks of 3: (k*3+j), j in (brk, str, thr)
NW = 2310

BF16 = ml_dtypes.bfloat16

_compiled = {}


def _build_fast(widths):
    """Bias-free program, software-pipelined across tiles.

    Each tile runs through 8 pipeline stages (speed-MLP x3, branch layers
    x3, heads, plus DMA lead-in); per tick every engine executes one stage
    of a different tile, so the in-order engine streams never stall on the
    intra-tile matmul->sigmoid chain.  All PSUM allocations rotate through
    one 4-slot pool (8 banks)."""
    import concourse.tile as tile
    from concourse import bacc, mybir

    f32 = mybir.dt.float32
    bf16 = mybir.dt.bfloat16
    Sig = mybir.ActivationFunctionType.Sigmoid
    Tanh = mybir.ActivationFunctionType.Tanh

    T = len(widths)
    nc = bacc.Bacc("TRN2", target_bir_lowering=False, debug=False,
                   num_devices=N_CORES)

    xT = nc.dram_tensor("xT", [T, 128, KX * RT], bf16, kind="ExternalInput")
    spdT = nc.dram_tensor("spdT", [T, 1, RT], bf16, kind="ExternalInput")
    Wt = nc.dram_tensor("Wt", [T, 128, NW], bf16, kind="ExternalInput")
    sw1 = nc.dram_tensor("sw1", [1, 128], bf16, kind="ExternalInput")
    sw23 = nc.dram_tensor("sw23", [128, 256], bf16, kind="ExternalInput")
    out = nc.dram_tensor("out", [T, 3, RT], f32, kind="ExternalOutput")

    with tile.TileContext(nc) as tc:
        with (
            tc.tile_pool(name="const", bufs=1) as cpool,
            tc.tile_pool(name="w", bufs=9) as wpool,
            tc.tile_pool(name="x", bufs=6) as xpool,
            tc.tile_pool(name="st", bufs=3) as stpool,
            tc.tile_pool(name="s", bufs=3) as spool,
            tc.tile_pool(name="h", bufs=3) as hpool,
            tc.tile_pool(name="o", bufs=2) as opool,
            tc.tile_pool(name="ps", bufs=3, space="PSUM") as pspool,
            tc.tile_pool(name="pss", bufs=2, space="PSUM") as psspool,
        ):
            sw1_t = cpool.tile([1, 128], bf16)
            nc.sync.dma_start(sw1_t[:], sw1[:])
            sw23_t = cpool.tile([128, 256], bf16)
            nc.sync.dma_start(sw23_t[:], sw23[:])

            st_t, s_t, h_t, xt_t, wt_t = {}, {}, {}, {}, {}

            def dma_st(t):
                w = widths[t]
                st = stpool.tile([1, RT], bf16, tag="st")
                nc.gpsimd.dma_start(st[:, :w], spdT[t][:, :w])
                st_t[t] = st

            def dma_xw(t):
                w = widths[t]
                xt = xpool.tile([128, KX * RT], bf16, tag="xt")
                nc.sync.dma_start(xt[:, :KX * w], xT[t][:, :KX * w])
                xt_t[t] = xt
                wt = wpool.tile([128, NW], bf16, tag="wt")
                nc.sync.dma_start(wt[:], Wt[t])
                wt_t[t] = wt

            def spd(t, stage):
                w = widths[t]
                ps = psspool.tile([128, RT], f32, tag="pss")
                if stage == 1:
                    nc.tensor.matmul(ps[:, :w], lhsT=sw1_t[:],
                                     rhs=st_t.pop(t)[:, :w],
                                     start=True, stop=True)
                else:
                    lhsT = (sw23_t[:, 0:128] if stage == 2
                            else sw23_t[:, 128:256])
                    nc.tensor.matmul(ps[:, :w], lhsT=lhsT,
                                     rhs=s_t.pop((t, stage - 1))[:, :w],
                                     start=True, stop=True)
                s = spool.tile([128, RT], bf16, tag=f"s{stage}")
                nc.scalar.activation(s[:, :w], ps[:, :w], Tanh)
                s_t[(t, stage)] = s

            def layer1(t):
                w = widths[t]
                wt, xt = wt_t[t], xt_t.pop(t)
                s3 = s_t.pop((t, 3))[:, :w]
                psm = pspool.tile([128, 2 * RT], f32, tag="ps")
                for m in range(2):
                    o = m * w
                    for k in range(KX):
                        nc.tensor.matmul(
                            psm[:, o:o + w],
                            lhsT=wt[:, (m * 5 + k) * 128:(m * 5 + k + 1) * 128],
                            rhs=xt[:, k * w:(k + 1) * w],
                            start=(k == 0), stop=False)
                    nc.tensor.matmul(
                        psm[:, o:o + w],
                        lhsT=wt[:, (m * 5 + 4) * 128:(m * 5 + 5) * 128],
                        rhs=s3, start=False, stop=True)
                h1 = hpool.tile([128, 2 * RT], bf16, tag="h1")
                nc.scalar.activation(h1[:, :2 * w], psm[:, :2 * w], Sig)
                h_t[(t, 1)] = h1

            def layer23(t, lay, off):
                w = widths[t]
                wt = wt_t[t]
                hin = h_t.pop((t, lay - 1))
                psm = pspool.tile([128, 2 * RT], f32, tag="ps")
                for m in range(2):
                    o = m * w
                    for k in range(2):
                        c0 = off + (m * 2 + k) * 128
                        nc.tensor.matmul(psm[:, o:o + w],
                                         lhsT=wt[:, c0:c0 + 128],
                                         rhs=hin[:, k * w:(k + 1) * w],
                                         start=(k == 0), stop=(k == 1))
                h = hpool.tile([128, 2 * RT], bf16, tag=f"h{lay}")
                nc.scalar.activation(h[:, :2 * w], psm[:, :2 * w], Sig)
                h_t[(t, lay)] = h

            def head(t):
                w = widths[t]
                wt = wt_t.pop(t)
                hin = h_t.pop((t, 3))
                php = pspool.tile([3, RT], f32, tag="ps")
                for k in range(2):
                    c0 = OFF_H + k * 3
                    nc.tensor.matmul(php[:, :w], lhsT=wt[:, c0:c0 + 3],
                                     rhs=hin[:, k * w:(k + 1) * w],
                                     start=(k == 0), stop=(k == 1))
                ot = opool.tile([3, RT], f32, tag="ot")
                nc.vector.tensor_copy(out=ot[:, :w], in_=php[:, :w])
                nc.gpsimd.dma_start(out[t][:, :w], ot[:, :w])

            def valid(t):
                return 0 <= t < T

            # tile stages: spd stages of tile t run at ticks
            # max(t-1,0)+1..+3 (one tick of slack ahead of L1, and tiles
            # 0/1 ramp concurrently); L1..L3 at t+4..t+6, heads at t+7.
            def spd_tiles(k, s):
                ts = []
                if 1 <= k - s + 1 < T:
                    ts.append(k - s + 1)
                if k == s:
                    ts.append(0)
                return sorted(set(ts))

            for k in range(T + 8):
                for t in ([0, 1] if k == 0 else ([k + 1] if k + 1 < T else [])):
                    dma_st(t)
                if valid(k - 7):
                    head(k - 7)
                if valid(k - 6):
                    layer23(k - 6, 3, OFF_W3)
                if valid(k - 5):
                    layer23(k - 5, 2, OFF_W2)
                if valid(k - 4):
                    layer1(k - 4)
                for t in spd_tiles(k, 3):
                    spd(t, 3)
                for t in spd_tiles(k, 2):
                    spd(t, 2)
                if valid(k):
                    dma_xw(k)
                for t in spd_tiles(k, 1):
                    spd(t, 1)

    nc.finalize()
    return nc


def _build_general(T):
    """General program with per-partition biases applied on ScalarE."""
    import concourse.tile as tile
    from concourse import bacc, mybir

    f32 = mybir.dt.float32
    bf16 = mybir.dt.bfloat16
    Sig = mybir.ActivationFunctionType.Sigmoid
    Tanh = mybir.ActivationFunctionType.Tanh

    nc = bacc.Bacc("TRN2", target_bir_lowering=False, debug=False,
                   num_devices=N_CORES)

    xT = nc.dram_tensor("xT", [T, 128, KX * RT], bf16, kind="ExternalInput")
    spdT = nc.dram_tensor("spdT", [T, 1, RT], bf16, kind="ExternalInput")
    Wt = nc.dram_tensor("Wt", [T, 128, NW], bf16, kind="ExternalInput")
    Bt = nc.dram_tensor("Bt", [T, 128, 8], f32, kind="ExternalInput")
    sw1 = nc.dram_tensor("sw1", [1, 128], bf16, kind="ExternalInput")
    sw23 = nc.dram_tensor("sw23", [128, 256], bf16, kind="ExternalInput")
    sbias = nc.dram_tensor("sbias", [128, 3], f32, kind="ExternalInput")
    out = nc.dram_tensor("out", [T, 3, RT], f32, kind="ExternalOutput")

    with tile.TileContext(nc) as tc:
        with (
            tc.tile_pool(name="const", bufs=1) as cpool,
            tc.tile_pool(name="w", bufs=3) as wpool,
            tc.tile_pool(name="b", bufs=3) as bpool,
            tc.tile_pool(name="x", bufs=3) as xpool,
            tc.tile_pool(name="s", bufs=3) as spool,
            tc.tile_pool(name="h", bufs=3) as hpool,
            tc.tile_pool(name="o", bufs=3) as opool,
            tc.tile_pool(name="ps", bufs=6, space="PSUM") as pspool,
            tc.tile_pool(name="psh", bufs=2, space="PSUM") as pshpool,
        ):
            sw1_t = cpool.tile([1, 128], bf16)
            nc.sync.dma_start(sw1_t[:], sw1[:])
            sw23_t = cpool.tile([128, 256], bf16)
            nc.sync.dma_start(sw23_t[:], sw23[:])
            sb_t = cpool.tile([128, 3], f32)
            nc.sync.dma_start(sb_t[:], sbias[:])

            for t in range(T):
                wt = wpool.tile([128, NW], bf16, tag="wt")
                nc.sync.dma_start(wt[:], Wt[t])
                bt = bpool.tile([128, 8], f32, tag="bt")
                nc.sync.dma_start(bt[:], Bt[t])
                xt = xpool.tile([128, KX * RT], bf16, tag="xt")
                nc.sync.dma_start(xt[:], xT[t])
                st = spool.tile([1, RT], bf16, tag="st")
                nc.sync.dma_start(st[:], spdT[t])

                ps = pspool.tile([128, RT], f32, tag="ps")
                nc.tensor.matmul(ps[:], lhsT=sw1_t[:], rhs=st[:],
                                 start=True, stop=True)
                s1 = spool.tile([128, RT], bf16, tag="s1")
                nc.scalar.activation(s1[:], ps[:], Tanh, bias=sb_t[:, 0:1])
                ps = pspool.tile([128, RT], f32, tag="ps")
                nc.tensor.matmul(ps[:], lhsT=sw23_t[:, 0:128], rhs=s1[:],
                                 start=True, stop=True)
                s2 = spool.tile([128, RT], bf16, tag="s2")
                nc.scalar.activation(s2[:], ps[:], Tanh, bias=sb_t[:, 1:2])
                ps = pspool.tile([128, RT], f32, tag="ps")
                nc.tensor.matmul(ps[:], lhsT=sw23_t[:, 128:256], rhs=s2[:],
                                 start=True, stop=True)
                s3 = spool.tile([128, RT], bf16, tag="s3")
                nc.scalar.activation(s3[:], ps[:], Tanh, bias=sb_t[:, 2:3])

                h1 = []
                for m in range(2):
                    psm = pspool.tile([128, RT], f32, tag="ps")
                    for k in range(KX):
                        nc.tensor.matmul(
                            psm[:], lhsT=wt[:, (m * 5 + k) * 128:(m * 5 + k + 1) * 128],
                            rhs=xt[:, k * RT:(k + 1) * RT],
                            start=(k == 0), stop=False)
                    nc.tensor.matmul(
                        psm[:], lhsT=wt[:, (m * 5 + 4) * 128:(m * 5 + 5) * 128],
                        rhs=s3, start=False, stop=True)
                    hm = hpool.tile([128, RT], bf16, tag=f"h1_{m}")
                    nc.scalar.activation(hm[:], psm[:], Sig, bias=bt[:, m:m + 1])
                    h1.append(hm)

                hin = h1
                for lay, off, bcol in ((2, OFF_W2, 2), (3, OFF_W3, 4)):
                    hs = []
                    for m in range(2):
                        psm = pspool.tile([128, RT], f32, tag="ps")
                        for k in range(2):
                            c0 = off + (m * 2 + k) * 128
                            nc.tensor.matmul(psm[:], lhsT=wt[:, c0:c0 + 128],
                                             rhs=hin[k][:],
                                             start=(k == 0), stop=(k == 1))
                        hm = hpool.tile([128, RT], bf16, tag=f"h{lay}_{m}")
                        nc.scalar.activation(hm[:], psm[:], Sig,
                                             bias=bt[:, bcol + m:bcol + m + 1])
                        hs.append(hm)
                    hin = hs

                php = pshpool.tile([3, RT], f32, tag="ph")
                for k in range(2):
                    c0 = OFF_H + k * 3
                    nc.tensor.matmul(php[:], lhsT=wt[:, c0:c0 + 3],
                                     rhs=hin[k][:], start=(k == 0), stop=(k == 1))
                ot = opool.tile([3, RT], f32, tag="ot")
                nc.vector.tensor_copy(out=ot[:], in_=php[:])
                nc.sync.dma_start(out[t], ot[:])

    nc.finalize()
    return nc


def kernel(rgb_feat, spd, cmd,
           spd_W1, spd_b1, spd_W2, spd_b2, spd_W3, spd_b3,
           W1, b1, W2, b2, W3, b3,
           hW_brk, hb_brk, hW_str, hb_str, hW_thr, hb_thr):
    from concourse.bass_utils import run_bass_kernel_spmd

    rgb_feat = np.asarray(rgb_feat, dtype=np.float32)
    spd = np.asarray(spd, dtype=np.float32)
    cmd = np.asarray(cmd, dtype=np.int32)
    W1 = np.asarray(W1, dtype=np.float32)
    W2 = np.asarray(W2, dtype=np.float32)
    W3 = np.asarray(W3, dtype=np.float32)
    b1 = np.asarray(b1, dtype=np.float32)
    b2 = np.asarray(b2, dtype=np.float32)
    b3 = np.asarray(b3, dtype=np.float32)
    sb1 = np.asarray(spd_b1, np.float32)
    sb2 = np.asarray(spd_b2, np.float32)
    sb3 = np.asarray(spd_b3, np.float32)
    hb_brk = np.asarray(hb_brk, np.float32)
    hb_str = np.asarray(hb_str, np.float32)
    hb_thr = np.asarray(hb_thr, np.float32)

    n_rows = rgb_feat.shape[0]
    zero_bias = not (b1.any() or b2.any() or b3.any() or sb1.any()
                     or sb2.any() or sb3.any() or hb_brk.any()
                     or hb_str.any() or hb_thr.any())

    # ---- host routing: per-branch rows into 512-row tiles, plus (fast
    # path only) one 256-row half tile per branch for small remainders ----
    RT2 = RT // 2
    branch_rows = [np.nonzero(cmd == c)[0] for c in range(C)]
    fill_c = int(np.argmax([len(r) for r in branch_rows]))
    full_tiles, half_tiles = [], []  # (branch, row_idx[w])
    for c in range(C):
        rows = branch_rows[c]
        if len(rows) == 0:
            continue
        a, rem = divmod(len(rows), RT)
        if rem == 0:
            n_f, n_h = a, 0
        elif zero_bias and rem <= RT2:
            n_f, n_h = a, 1
        else:
            n_f, n_h = a + 1, 0
        cap = n_f * RT + n_h * RT2
        rows_p = np.concatenate([rows, np.full(cap - len(rows), rows[0],
                                               np.int64)])
        for i in range(n_f):
            full_tiles.append((c, rows_p[i * RT:(i + 1) * RT]))
        if n_h:
            half_tiles.append((c, rows_p[n_f * RT:]))
    TF = math.ceil(len(full_tiles) / N_CORES)
    TH = math.ceil(len(half_tiles) / N_CORES)
    while len(full_tiles) < TF * N_CORES:
        full_tiles.append((fill_c, np.full(RT, branch_rows[fill_c][0],
                                           np.int64)))
    while len(half_tiles) < TH * N_CORES:
        half_tiles.append((fill_c, np.full(RT2, branch_rows[fill_c][0],
                                           np.int64)))
    widths = [RT] * TF + [RT2] * TH
    T = TF + TH
    core_tiles = [full_tiles[i * TF:(i + 1) * TF]
                  + half_tiles[i * TH:(i + 1) * TH] for i in range(N_CORES)]

    # ---- pack per-branch weights once (bf16) and biases (fp32) ----
    wt_b = np.empty((C, 128, NW), BF16)
    bt_b = np.empty((C, 128, 8), np.float32)
    for c in range(C):
        w = wt_b[c]
        for m in range(2):
            for k in range(5):
                w[:, (m * 5 + k) * 128:(m * 5 + k + 1) * 128] = \
                    W1[c, k * 128:(k + 1) * 128, m * 128:(m + 1) * 128]
            for k in range(2):
                c0 = OFF_W2 + (m * 2 + k) * 128
                w[:, c0:c0 + 128] = W2[c, k * 128:(k + 1) * 128,
                                       m * 128:(m + 1) * 128]
                c0 = OFF_W3 + (m * 2 + k) * 128
                w[:, c0:c0 + 128] = W3[c, k * 128:(k + 1) * 128,
                                       m * 128:(m + 1) * 128]
        for k in range(2):
            c0 = OFF_H + k * 3
            w[:, c0 + 0] = hW_brk[c, k * 128:(k + 1) * 128, 0]
            w[:, c0 + 1] = hW_str[c, k * 128:(k + 1) * 128, 0]
            w[:, c0 + 2] = hW_thr[c, k * 128:(k + 1) * 128, 0]
        bb = bt_b[c]
        bb[:] = 0.0
        bb[:, 0] = b1[c, 0:128]
        bb[:, 1] = b1[c, 128:256]
        bb[:, 2] = b2[c, 0:128]
        bb[:, 3] = b2[c, 128:256]
        bb[:, 4] = b3[c, 0:128]
        bb[:, 5] = b3[c, 128:256]
        bb[0, 6] = hb_brk[c, 0]
        bb[1, 6] = 2.0 * hb_str[c, 0]  # tanh(z) = 2*sigmoid(2z)-1
        bb[2, 6] = hb_thr[c, 0]
        bb[0, 7] = 1.0
        bb[1, 7] = 2.0
        bb[2, 7] = 1.0

    sw1_h = np.asarray(spd_W1, np.float32).astype(BF16)  # [1,128]
    sw23_h = np.empty((128, 256), BF16)
    sw23_h[:, 0:128] = np.asarray(spd_W2, np.float32)
    sw23_h[:, 128:256] = np.asarray(spd_W3, np.float32)
    sb_h = np.stack([sb1, sb2, sb3], axis=1)  # [128,3]

    # ---- build per-core inputs ----
    in_maps = []
    for i in range(N_CORES):
        my = core_tiles[i]
        xT_h = np.zeros((T, 128, KX * RT), BF16)
        spdT_h = np.zeros((T, 1, RT), BF16)
        for t, (c, rows_t) in enumerate(my):
            w = widths[t]
            g = rgb_feat[rows_t]  # [w, 512]
            xT_h[t, :, :KX * w] = (
                g.reshape(w, KX, 128).transpose(2, 1, 0).reshape(128, KX * w))
            spdT_h[t, 0, :w] = spd[rows_t, 0]
        Wt_h = np.ascontiguousarray(wt_b[[c for c, _ in my]])
        m = {
            "xT": xT_h, "spdT": spdT_h, "Wt": Wt_h,
            "sw1": sw1_h, "sw23": sw23_h,
        }
        if not zero_bias:
            m["Bt"] = np.ascontiguousarray(bt_b[[c for c, _ in my]])
            m["sbias"] = sb_h
        in_maps.append(m)

    key = (tuple(widths), zero_bias)
    if key not in _compiled:
        _compiled[key] = (_build_fast(widths) if zero_bias
                          else _build_general(T))
    nc = _compiled[key]

    res = run_bass_kernel_spmd(nc, in_maps, core_ids=list(range(N_CORES)))

    # ---- head bias + activations on host, then unscatter ----
    np_err = np.seterr(over="ignore", under="ignore")
    # (duplicate rows write identical values)
    brake = np.empty(n_rows, np.float32)
    steer = np.empty(n_rows, np.float32)
    throttle = np.empty(n_rows, np.float32)
    for i in range(N_CORES):
        o = res.results[i]["out"].astype(np.float64)  # [T, 3, RT] pre-act z
        for t, (c, rows_t) in enumerate(core_tiles[i]):
            w = widths[t]
            brake[rows_t] = 1.0 / (1.0 + np.exp(-(o[t, 0, :w] + hb_brk[c, 0])))
            steer[rows_t] = np.tanh(o[t, 1, :w] + hb_str[c, 0])
            throttle[rows_t] = 1.0 / (1.0 + np.exp(-(o[t, 2, :w]
                                                     + hb_thr[c, 0])))
    np.seterr(**np_err)
    return brake[:, None], steer[:, None], throttle[:, None]


# revision 20
# speedup vs baseline: 1.0140x; 1.0140x over previous
"""Trainium2 Bass kernel for the branched (command-routed) control MLP.

Model: sfeat = tanh-MLP(spd) [B,128]; x = [rgb_feat | sfeat] [B,640];
per-row branch b = cmd[r] selects one of 7 MLPs 640->256->256->256 (sigmoid)
and 3 scalar heads (sigmoid/tanh/sigmoid).

Strategy (MoE routing): rows are sorted by cmd on the host and packed into
single-branch tiles (512 rows, plus one 256-row tile per core for branch
remainders); tiles are distributed over 8 NeuronCores.  Each tile's branch
weights are shipped per-tile (bf16), so the device program is identical on
every core (SPMD) while computing only the selected branch per row: 7x less
matmul work than the all-branch reference.

Device: software-pipelined across tiles (8 stages: DMA, speed-MLP x3,
branch layers x3, heads), so the in-order PE/ScalarE streams never stall on
the intra-tile matmul->sigmoid chain.  Branch layers run as K-accumulated
bf16 matmuls into fp32 PSUM with one [128,1024] sigmoid per layer per tile
(both 128-row H chunks in one ScalarE op); the 3 heads are one [K=256,M=3]
matmul whose fp32 pre-activations go back via an idle-VectorE copy + DMA.
The host adds head biases, applies the final sigmoid/tanh exactly, and
scatters tile outputs to original row order; padded rows are duplicates of
same-branch rows so duplicate scatters write identical values.

Two compiled variants: a fast bias-free path (all biases in this model's
init are zero) and a general path applying per-partition biases on ScalarE.
Measured on 8 axon trn2 cores: ~77 us HW exec (vs ~104 us for the same
mapping without software pipelining).
"""

import sys

if "/opt/trn_rl_repo" not in sys.path:
    sys.path.insert(0, "/opt/trn_rl_repo")

import math

import ml_dtypes
import numpy as np

B = 32768
D_RGB = 512
D_SPD = 128
D_IN = 640  # 5 * 128
H = 256
C = 7
N_CORES = 8
RT = 512  # rows per tile
KX = 4  # rgb K-chunks of 128

# packed weight column offsets (bf16 [128, NW] per tile)
OFF_W1 = 0  # 10 blocks of 128: (m*5+k)
OFF_W2 = 1280  # 4 blocks of 128: (m*2+k)
OFF_W3 = 1792  # 4 blocks of 128
OFF_H = 2304  # 2 blocks of 3: (k*3+j), j in (brk, str, thr)
NW = 2310

BF16 = ml_dtypes.bfloat16

_compiled = {}


def _build_fast(widths):
    """Bias-free program, software-pipelined across tiles.

    Each tile runs through 8 pipeline stages (speed-MLP x3, branch layers
    x3, heads, plus DMA lead-in); per tick every engine executes one stage
    of a different tile, so the in-order engine streams never stall on the
    intra-tile matmul->sigmoid chain.  All PSUM allocations rotate through
    one 4-slot pool (8 banks)."""
    import concourse.tile as tile
    from concourse import bacc, mybir

    f32 = mybir.dt.float32
    bf16 = mybir.dt.bfloat16
    Sig = mybir.ActivationFunctionType.Sigmoid
    Tanh = mybir.ActivationFunctionType.Tanh

    T = len(widths)
    nc = bacc.Bacc("TRN2", target_bir_lowering=False, debug=False,
                   num_devices=N_CORES)

    xT = nc.dram_tensor("xT", [T, 128, KX * RT], bf16, kind="ExternalInput")
    spdT = nc.dram_tensor("spdT", [T, 1, RT], bf16, kind="ExternalInput")
    Wt = nc.dram_tensor("Wt", [T, 128, NW], bf16, kind="ExternalInput")
    sw1 = nc.dram_tensor("sw1", [1, 128], bf16, kind="ExternalInput")
    sw23 = nc.dram_tensor("sw23", [128, 256], bf16, kind="ExternalInput")
    out = nc.dram_tensor("out", [T, 3, RT], f32, kind="ExternalOutput")

    with tile.TileContext(nc) as tc:
        with (
            tc.tile_pool(name="const", bufs=1) as cpool,
            tc.tile_pool(name="w", bufs=9) as wpool,
            tc.tile_pool(name="x", bufs=6) as xpool,
            tc.tile_pool(name="st", bufs=2) as stpool,
            tc.tile_pool(name="s", bufs=2) as spool,
            tc.tile_pool(name="h", bufs=3) as hpool,
            tc.tile_pool(name="o", bufs=2) as opool,
            tc.tile_pool(name="ps", bufs=3, space="PSUM") as pspool,
            tc.tile_pool(name="pss", bufs=2, space="PSUM") as psspool,
        ):
            sw1_t = cpool.tile([1, 128], bf16)
            nc.sync.dma_start(sw1_t[:], sw1[:])
            sw23_t = cpool.tile([128, 256], bf16)
            nc.sync.dma_start(sw23_t[:], sw23[:])

            st_t, s_t, h_t, xt_t, wt_t = {}, {}, {}, {}, {}

            def dma_st(t):
                w = widths[t]
                st = stpool.tile([1, RT], bf16, tag="st")
                nc.gpsimd.dma_start(st[:, :w], spdT[t][:, :w])
                st_t[t] = st

            def dma_xw(t):
                w = widths[t]
                xt = xpool.tile([128, KX * RT], bf16, tag="xt")
                nc.sync.dma_start(xt[:, :KX * w], xT[t][:, :KX * w])
                xt_t[t] = xt
                wt = wpool.tile([128, NW], bf16, tag="wt")
                nc.sync.dma_start(wt[:], Wt[t])
                wt_t[t] = wt

            def spd(t, stage):
                w = widths[t]
                ps = psspool.tile([128, RT], f32, tag="pss")
                if stage == 1:
                    nc.tensor.matmul(ps[:, :w], lhsT=sw1_t[:],
                                     rhs=st_t.pop(t)[:, :w],
                                     start=True, stop=True)
                else:
                    lhsT = (sw23_t[:, 0:128] if stage == 2
                            else sw23_t[:, 128:256])
                    nc.tensor.matmul(ps[:, :w], lhsT=lhsT,
                                     rhs=s_t.pop((t, stage - 1))[:, :w],
                                     start=True, stop=True)
                s = spool.tile([128, RT], bf16, tag=f"s{stage}")
                nc.scalar.activation(s[:, :w], ps[:, :w], Tanh)
                s_t[(t, stage)] = s

            def layer1(t):
                w = widths[t]
                wt, xt = wt_t[t], xt_t.pop(t)
                s3 = s_t.pop((t, 3))[:, :w]
                psm = pspool.tile([128, 2 * RT], f32, tag="ps")
                for m in range(2):
                    o = m * w
                    for k in range(KX):
                        nc.tensor.matmul(
                            psm[:, o:o + w],
                            lhsT=wt[:, (m * 5 + k) * 128:(m * 5 + k + 1) * 128],
                            rhs=xt[:, k * w:(k + 1) * w],
                            start=(k == 0), stop=False)
                    nc.tensor.matmul(
                        psm[:, o:o + w],
                        lhsT=wt[:, (m * 5 + 4) * 128:(m * 5 + 5) * 128],
                        rhs=s3, start=False, stop=True)
                h1 = hpool.tile([128, 2 * RT], bf16, tag="h1")
                nc.scalar.activation(h1[:, :2 * w], psm[:, :2 * w], Sig)
                h_t[(t, 1)] = h1

            def layer23(t, lay, off):
                w = widths[t]
                wt = wt_t[t]
                hin = h_t.pop((t, lay - 1))
                psm = pspool.tile([128, 2 * RT], f32, tag="ps")
                for m in range(2):
                    o = m * w
                    for k in range(2):
                        c0 = off + (m * 2 + k) * 128
                        nc.tensor.matmul(psm[:, o:o + w],
                                         lhsT=wt[:, c0:c0 + 128],
                                         rhs=hin[:, k * w:(k + 1) * w],
                                         start=(k == 0), stop=(k == 1))
                h = hpool.tile([128, 2 * RT], bf16, tag=f"h{lay}")
                nc.scalar.activation(h[:, :2 * w], psm[:, :2 * w], Sig)
                h_t[(t, lay)] = h

            def head(t):
                w = widths[t]
                wt = wt_t.pop(t)
                hin = h_t.pop((t, 3))
                php = pspool.tile([3, RT], f32, tag="ps")
                for k in range(2):
                    c0 = OFF_H + k * 3
                    nc.tensor.matmul(php[:, :w], lhsT=wt[:, c0:c0 + 3],
                                     rhs=hin[:, k * w:(k + 1) * w],
                                     start=(k == 0), stop=(k == 1))
                ot = opool.tile([3, RT], f32, tag="ot")
                nc.vector.tensor_copy(out=ot[:, :w], in_=php[:, :w])
                nc.gpsimd.dma_start(out[t][:, :w], ot[:, :w])

            def valid(t):
                return 0 <= t < T

            # tile stages: st/x/w-DMA at t, spd1..3 at t+1..t+3,
            # L1..L3 at t+4..t+6, heads at t+7; deepest-first emission.
            for k in range(T + 8):
                if valid(k):
                    dma_st(k)
                if valid(k - 7):
                    head(k - 7)
                if valid(k - 6):
                    layer23(k - 6, 3, OFF_W3)
                if valid(k - 5):
                    layer23(k - 5, 2, OFF_W2)
                if valid(k - 4):
                    layer1(k - 4)
                if valid(k - 3):
                    spd(k - 3, 3)
                if valid(k - 2):
                    spd(k - 2, 2)
                if valid(k):
                    dma_xw(k)
                if valid(k - 1):
                    spd(k - 1, 1)

    nc.finalize()
    return nc


def _build_general(T):
    """General program with per-partition biases applied on ScalarE."""
    import concourse.tile as tile
    from concourse import bacc, mybir

    f32 = mybir.dt.float32
    bf16 = mybir.dt.bfloat16
    Sig = mybir.ActivationFunctionType.Sigmoid
    Tanh = mybir.ActivationFunctionType.Tanh

    nc = bacc.Bacc("TRN2", target_bir_lowering=False, debug=False,
                   num_devices=N_CORES)

    xT = nc.dram_tensor("xT", [T, 128, KX * RT], bf16, kind="ExternalInput")
    spdT = nc.dram_tensor("spdT", [T, 1, RT], bf16, kind="ExternalInput")
    Wt = nc.dram_tensor("Wt", [T, 128, NW], bf16, kind="ExternalInput")
    Bt = nc.dram_tensor("Bt", [T, 128, 8], f32, kind="ExternalInput")
    sw1 = nc.dram_tensor("sw1", [1, 128], bf16, kind="ExternalInput")
    sw23 = nc.dram_tensor("sw23", [128, 256], bf16, kind="ExternalInput")
    sbias = nc.dram_tensor("sbias", [128, 3], f32, kind="ExternalInput")
    out = nc.dram_tensor("out", [T, 3, RT], f32, kind="ExternalOutput")

    with tile.TileContext(nc) as tc:
        with (
            tc.tile_pool(name="const", bufs=1) as cpool,
            tc.tile_pool(name="w", bufs=3) as wpool,
            tc.tile_pool(name="b", bufs=3) as bpool,
            tc.tile_pool(name="x", bufs=3) as xpool,
            tc.tile_pool(name="s", bufs=2) as spool,
            tc.tile_pool(name="h", bufs=3) as hpool,
            tc.tile_pool(name="o", bufs=3) as opool,
            tc.tile_pool(name="ps", bufs=6, space="PSUM") as pspool,
            tc.tile_pool(name="psh", bufs=2, space="PSUM") as pshpool,
        ):
            sw1_t = cpool.tile([1, 128], bf16)
            nc.sync.dma_start(sw1_t[:], sw1[:])
            sw23_t = cpool.tile([128, 256], bf16)
            nc.sync.dma_start(sw23_t[:], sw23[:])
            sb_t = cpool.tile([128, 3], f32)
            nc.sync.dma_start(sb_t[:], sbias[:])

            for t in range(T):
                wt = wpool.tile([128, NW], bf16, tag="wt")
                nc.sync.dma_start(wt[:], Wt[t])
                bt = bpool.tile([128, 8], f32, tag="bt")
                nc.sync.dma_start(bt[:], Bt[t])
                xt = xpool.tile([128, KX * RT], bf16, tag="xt")
                nc.sync.dma_start(xt[:], xT[t])
                st = spool.tile([1, RT], bf16, tag="st")
                nc.sync.dma_start(st[:], spdT[t])

                ps = pspool.tile([128, RT], f32, tag="ps")
                nc.tensor.matmul(ps[:], lhsT=sw1_t[:], rhs=st[:],
                                 start=True, stop=True)
                s1 = spool.tile([128, RT], bf16, tag="s1")
                nc.scalar.activation(s1[:], ps[:], Tanh, bias=sb_t[:, 0:1])
                ps = pspool.tile([128, RT], f32, tag="ps")
                nc.tensor.matmul(ps[:], lhsT=sw23_t[:, 0:128], rhs=s1[:],
                                 start=True, stop=True)
                s2 = spool.tile([128, RT], bf16, tag="s2")
                nc.scalar.activation(s2[:], ps[:], Tanh, bias=sb_t[:, 1:2])
                ps = pspool.tile([128, RT], f32, tag="ps")
                nc.tensor.matmul(ps[:], lhsT=sw23_t[:, 128:256], rhs=s2[:],
                                 start=True, stop=True)
                s3 = spool.tile([128, RT], bf16, tag="s3")
                nc.scalar.activation(s3[:], ps[:], Tanh, bias=sb_t[:, 2:3])

                h1 = []
                for m in range(2):
                    psm = pspool.tile([128, RT], f32, tag="ps")
                    for k in range(KX):
                        nc.tensor.matmul(
                            psm[:], lhsT=wt[:, (m * 5 + k) * 128:(m * 5 + k + 1) * 128],
                            rhs=xt[:, k * RT:(k + 1) * RT],
                            start=(k == 0), stop=False)
                    nc.tensor.matmul(
                        psm[:], lhsT=wt[:, (m * 5 + 4) * 128:(m * 5 + 5) * 128],
                        rhs=s3, start=False, stop=True)
                    hm = hpool.tile([128, RT], bf16, tag=f"h1_{m}")
                    nc.scalar.activation(hm[:], psm[:], Sig, bias=bt[:, m:m + 1])
                    h1.append(hm)

                hin = h1
                for lay, off, bcol in ((2, OFF_W2, 2), (3, OFF_W3, 4)):
                    hs = []
                    for m in range(2):
                        psm = pspool.tile([128, RT], f32, tag="ps")
                        for k in range(2):
                            c0 = off + (m * 2 + k) * 128
                            nc.tensor.matmul(psm[:], lhsT=wt[:, c0:c0 + 128],
                                             rhs=hin[k][:],
                                             start=(k == 0), stop=(k == 1))
                        hm = hpool.tile([128, RT], bf16, tag=f"h{lay}_{m}")
                        nc.scalar.activation(hm[:], psm[:], Sig,
                                             bias=bt[:, bcol + m:bcol + m + 1])
                        hs.append(hm)
                    hin = hs

                php = pshpool.tile([3, RT], f32, tag="ph")
                for k in range(2):
                    c0 = OFF_H + k * 3
                    nc.tensor.matmul(php[:], lhsT=wt[:, c0:c0 + 3],
                                     rhs=hin[k][:], start=(k == 0), stop=(k == 1))
                ot = opool.tile([3, RT], f32, tag="ot")
                nc.vector.tensor_copy(out=ot[:], in_=php[:])
                nc.sync.dma_start(out[t], ot[:])

    nc.finalize()
    return nc


def kernel(rgb_feat, spd, cmd,
           spd_W1, spd_b1, spd_W2, spd_b2, spd_W3, spd_b3,
           W1, b1, W2, b2, W3, b3,
           hW_brk, hb_brk, hW_str, hb_str, hW_thr, hb_thr):
    from concourse.bass_utils import run_bass_kernel_spmd

    rgb_feat = np.asarray(rgb_feat, dtype=np.float32)
    spd = np.asarray(spd, dtype=np.float32)
    cmd = np.asarray(cmd, dtype=np.int32)
    W1 = np.asarray(W1, dtype=np.float32)
    W2 = np.asarray(W2, dtype=np.float32)
    W3 = np.asarray(W3, dtype=np.float32)
    b1 = np.asarray(b1, dtype=np.float32)
    b2 = np.asarray(b2, dtype=np.float32)
    b3 = np.asarray(b3, dtype=np.float32)
    sb1 = np.asarray(spd_b1, np.float32)
    sb2 = np.asarray(spd_b2, np.float32)
    sb3 = np.asarray(spd_b3, np.float32)
    hb_brk = np.asarray(hb_brk, np.float32)
    hb_str = np.asarray(hb_str, np.float32)
    hb_thr = np.asarray(hb_thr, np.float32)

    n_rows = rgb_feat.shape[0]
    zero_bias = not (b1.any() or b2.any() or b3.any() or sb1.any()
                     or sb2.any() or sb3.any() or hb_brk.any()
                     or hb_str.any() or hb_thr.any())

    # ---- host routing: per-branch rows into 512-row tiles, plus (fast
    # path only) one 256-row half tile per branch for small remainders ----
    RT2 = RT // 2
    branch_rows = [np.nonzero(cmd == c)[0] for c in range(C)]
    fill_c = int(np.argmax([len(r) for r in branch_rows]))
    full_tiles, half_tiles = [], []  # (branch, row_idx[w])
    for c in range(C):
        rows = branch_rows[c]
        if len(rows) == 0:
            continue
        a, rem = divmod(len(rows), RT)
        if rem == 0:
            n_f, n_h = a, 0
        elif zero_bias and rem <= RT2:
            n_f, n_h = a, 1
        else:
            n_f, n_h = a + 1, 0
        cap = n_f * RT + n_h * RT2
        rows_p = np.concatenate([rows, np.full(cap - len(rows), rows[0],
                                               np.int64)])
        for i in range(n_f):
            full_tiles.append((c, rows_p[i * RT:(i + 1) * RT]))
        if n_h:
            half_tiles.append((c, rows_p[n_f * RT:]))
    TF = math.ceil(len(full_tiles) / N_CORES)
    TH = math.ceil(len(half_tiles) / N_CORES)
    while len(full_tiles) < TF * N_CORES:
        full_tiles.append((fill_c, np.full(RT, branch_rows[fill_c][0],
                                           np.int64)))
    while len(half_tiles) < TH * N_CORES:
        half_tiles.append((fill_c, np.full(RT2, branch_rows[fill_c][0],
                                           np.int64)))
    widths = [RT] * TF + [RT2] * TH
    T = TF + TH
    core_tiles = [full_tiles[i * TF:(i + 1) * TF]
                  + half_tiles[i * TH:(i + 1) * TH] for i in range(N_CORES)]

    # ---- pack per-branch weights once (bf16) and biases (fp32) ----
    wt_b = np.empty((C, 128, NW), BF16)
    bt_b = np.empty((C, 128, 8), np.float32)
    for c in range(C):
        w = wt_b[c]
        for m in range(2):
            for k in range(5):
                w[:, (m * 5 + k) * 128:(m * 5 + k + 1) * 128] = \
                    W1[c, k * 128:(k + 1) * 128, m * 128:(m + 1) * 128]
            for k in range(2):
                c0 = OFF_W2 + (m * 2 + k) * 128
                w[:, c0:c0 + 128] = W2[c, k * 128:(k + 1) * 128,
                                       m * 128:(m + 1) * 128]
                c0 = OFF_W3 + (m * 2 + k) * 128
                w[:, c0:c0 + 128] = W3[c, k * 128:(k + 1) * 128,
                                       m * 128:(m + 1) * 128]
        for k in range(2):
            c0 = OFF_H + k * 3
            w[:, c0 + 0] = hW_brk[c, k * 128:(k + 1) * 128, 0]
            w[:, c0 + 1] = hW_str[c, k * 128:(k + 1) * 128, 0]
            w[:, c0 + 2] = hW_thr[c, k * 128:(k + 1) * 128, 0]
        bb = bt_b[c]
        bb[:] = 0.0
        bb[:, 0] = b1[c, 0:128]
        bb[:, 1] = b1[c, 128:256]
        bb[:, 2] = b2[c, 0:128]
        bb[:, 3] = b2[c, 128:256]
        bb[:, 4] = b3[c, 0:128]
        bb[:, 5] = b3[c, 128:256]
        bb[0, 6] = hb_brk[c, 0]
        bb[1, 6] = 2.0 * hb_str[c, 0]  # tanh(z) = 2*sigmoid(2z)-1
        bb[2, 6] = hb_thr[c, 0]
        bb[0, 7] = 1.0
        bb[1, 7] = 2.0
        bb[2, 7] = 1.0

    sw1_h = np.asarray(spd_W1, np.float32).astype(BF16)  # [1,128]
    sw23_h = np.empty((128, 256), BF16)
    sw23_h[:, 0:128] = np.asarray(spd_W2, np.float32)
    sw23_h[:, 128:256] = np.asarray(spd_W3, np.float32)
    sb_h = np.stack([sb1, sb2, sb3], axis=1)  # [128,3]

    # ---- build per-core inputs ----
    in_maps = []
    for i in range(N_CORES):
        my = core_tiles[i]
        xT_h = np.zeros((T, 128, KX * RT), BF16)
        spdT_h = np.zeros((T, 1, RT), BF16)
        for t, (c, rows_t) in enumerate(my):
            w = widths[t]
            g = rgb_feat[rows_t]  # [w, 512]
            xT_h[t, :, :KX * w] = (
                g.reshape(w, KX, 128).transpose(2, 1, 0).reshape(128, KX * w))
            spdT_h[t, 0, :w] = spd[rows_t, 0]
        Wt_h = np.ascontiguousarray(wt_b[[c for c, _ in my]])
        m = {
            "xT": xT_h, "spdT": spdT_h, "Wt": Wt_h,
            "sw1": sw1_h, "sw23": sw23_h,
        }
        if not zero_bias:
            m["Bt"] = np.ascontiguousarray(bt_b[[c for c, _ in my]])
            m["sbias"] = sb_h
        in_maps.append(m)

    key = (tuple(widths), zero_bias)
    if key not in _compiled:
        _compiled[key] = (_build_fast(widths) if zero_bias
                          else _build_general(T))
    nc = _compiled[key]

    res = run_bass_kernel_spmd(nc, in_maps, core_ids=list(range(N_CORES)))

    # ---- head bias + activations on host, then unscatter ----
    np_err = np.seterr(over="ignore", under="ignore")
    # (duplicate rows write identical values)
    brake = np.empty(n_rows, np.float32)
    steer = np.empty(n_rows, np.float32)
    throttle = np.empty(n_rows, np.float32)
    for i in range(N_CORES):
        o = res.results[i]["out"].astype(np.float64)  # [T, 3, RT] pre-act z
        for t, (c, rows_t) in enumerate(core_tiles[i]):
            w = widths[t]
            brake[rows_t] = 1.0 / (1.0 + np.exp(-(o[t, 0, :w] + hb_brk[c, 0])))
            steer[rows_t] = np.tanh(o[t, 1, :w] + hb_str[c, 0])
            throttle[rows_t] = 1.0 / (1.0 + np.exp(-(o[t, 2, :w]
                                                     + hb_thr[c, 0])))
    np.seterr(**np_err)
    return brake[:, None], steer[:, None], throttle[:, None]
